# revision 10
# baseline (speedup 1.0000x reference)
"""DCRNN kernel for Trainium2 (8 NeuronCores, data-parallel over batch).

Model (per time step t, 6 steps):
    z  = relu([x_t, h] @ fc_w.T + fc_b)          # [b, n, 128]
    zd = einsum('nm,bmh->bnh', adj, z)           # graph diffusion
    GRU(zd, h) -> h                              # gated update
Readout: y = h @ out_w.T + out_b                 # [b, n, 714]

v3 design (per core, batch shard of 8, tokens packed TIGHT = 8*714 = 5712):
  - fp8 DoubleRow everywhere the contraction allows C=256 fusion:
      fc:   [h8; x8] planes vs [fc_wh x1024; fc_wx x64 + bias row] slots
      GRU:  r/u gates fused (ih+hh) via [zd8; h8] planes
      diffusion: z.T fp8 chunks vs adj fp8 (as v2)
  - scales: zd8 = zd*64, h8 = h*1, x8 = x*16, all gate PSUMs = 1024x true
  - i_n: mixed-dtype matmul (fp8 zd8 moving x bf16 w_in*16 stationary)
  - t1 = (h_n + b_hn)*r accumulated into i_n's PSUM via identity matmul;
    tanh reads PSUM directly (no sg2 DVE op)
  - h' = h + v*(c-h): d/e on DVE, h-copy + accumulate + h8-cast on the
    gpsimd DMA queue (0 DVE cost)
  - token pairs of 960/912 (16B-aligned for fp8 DR), matmuls split 512+rest
    at PSUM bank boundaries, sigma/tanh/relu ACTs span 2 banks (one op/pair)
  - readout flipped: yT[j, tok] with out_w slices stationary (LDW amortized),
    plain copies (bias added on host), DMAs spread across step-5 pairs
"""
import sys
import types

sys.path.insert(0, "/opt/trn_rl_repo")

import numpy as np
import ml_dtypes
from contextlib import ExitStack

# NTFF profile hook shim: the agent image lacks antenv.axon_hooks; provide it
# so run_bass_kernel_spmd(trace=True) can profile. Harmless when unused.
try:
    import antenv.axon_hooks  # noqa: F401
except ImportError:
    try:
        import trn_agent_boot.trn_boot as _tb

        _m = types.ModuleType("antenv.axon_hooks")
        _hook = _tb._ntff_profile_via_ctypes("/opt/axon/libaxon_pjrt.so")
        _m.get_axon_ntff_profile_hook = lambda: _hook
        _m.set_axon_ntff_profile_hook = lambda h: None
        sys.modules["antenv.axon_hooks"] = _m
    except Exception:
        pass

from concourse import bacc, tile, mybir
from concourse.bass_utils import run_bass_kernel_spmd

F32 = mybir.dt.float32
BF16 = mybir.dt.bfloat16
FP8 = mybir.dt.float8e4
AF = mybir.ActivationFunctionType
ALU = mybir.AluOpType
DR = mybir.MatmulPerfMode.DoubleRow

B, T, N, D, HID = 64, 6, 714, 16, 128
CORES = 8
BL = B // CORES            # batch per core
TOK = BL * N               # 5712 tight tokens
TOKP = TOK + 64            # z_fm pad tail so window-7 transpose stays in-bounds
NPAD8 = 720                # adj_dr inner pad (16-aligned)
NCH = 6                    # m-chunks per batch item (5x128 + 74)

# token pairs: 16B-aligned starts, PSUM 2-bank (1024 f32) tiles
PO = [0, 960, 1920, 2880, 3840, 4800]
PS = [960, 960, 960, 960, 960, 912]
NPAIR = 6

S_ADJ = 1024.0             # adj stored x1024 in fp8
SZ = 16.0                  # z stored x16 (z_fm bf16 and zt8 fp8)
SZD = 64.0                 # zd stored x64 in fp8
SWI = 16.0                 # w_ih (r/u/n slots vs zd8) x16
SWH = 1024.0               # w_hh (vs h8 x1) x1024
SX = 16.0                  # x plane x16
SWX = 64.0                 # fc x-weights x64
KG = 1024.0                # every gate PSUM = KG * true

_NC_CACHE = {}


def _build_program():
    if "nc" in _NC_CACHE:
        return _NC_CACHE["nc"]

    nc = bacc.Bacc(
        "TRN2",
        target_bir_lowering=False,
        debug=False,
        enable_asserts=True,
        num_devices=CORES,
    )

    x8_d = nc.declare_dram_parameter("x8", [T - 1, 17, TOK], FP8, isOutput=False)
    zd0_d = nc.declare_dram_parameter("zd0", [HID, TOK], FP8, isOutput=False)
    adjdr_d = nc.declare_dram_parameter("adj_dr", [128, 3, 2, NPAD8], FP8, isOutput=False)
    w2r_d = nc.declare_dram_parameter("w2r", [128, 2, 128], FP8, isOutput=False)
    w2u_d = nc.declare_dram_parameter("w2u", [128, 2, 128], FP8, isOutput=False)
    wir_d = nc.declare_dram_parameter("wir8", [128, 128], FP8, isOutput=False)
    wiu_d = nc.declare_dram_parameter("wiu8", [128, 128], FP8, isOutput=False)
    win_d = nc.declare_dram_parameter("w_inT", [128, 128], BF16, isOutput=False)
    whn_d = nc.declare_dram_parameter("w_hhnT", [128, 128], BF16, isOutput=False)
    fcw2_d = nc.declare_dram_parameter("fcw2", [128, 2, 128], FP8, isOutput=False)
    ident_d = nc.declare_dram_parameter("ident", [128, 128], BF16, isOutput=False)
    outw_d = nc.declare_dram_parameter("out_wT6", [128, 6, 128], BF16, isOutput=False)
    br_d = nc.declare_dram_parameter("b_r", [HID, 1], F32, isOutput=False)
    bun_d = nc.declare_dram_parameter("b_u_neg", [HID, 1], F32, isOutput=False)
    bin_d = nc.declare_dram_parameter("b_in", [HID, 1], F32, isOutput=False)
    bhn_d = nc.declare_dram_parameter("b_hn_s", [HID, 1], F32, isOutput=False)
    y_d = nc.declare_dram_parameter("yT", [N, TOK], BF16, isOutput=True)

    with tile.TileContext(nc) as tc, ExitStack() as ctx:
        cst = ctx.enter_context(tc.tile_pool(name="cst", bufs=1))
        st = ctx.enter_context(tc.tile_pool(name="st", bufs=1))
        gb = ctx.enter_context(tc.tile_pool(name="gb", bufs=4))
        ysb_p = ctx.enter_context(tc.tile_pool(name="ysb", bufs=8))
        ps = ctx.enter_context(tc.tile_pool(name="ps", bufs=3, space="PSUM"))

        # ---- constants in (t=0-critical first) ----
        zd0 = None  # DMA'd into S3 plane later (need tiles declared first)
        w2r = cst.tile([128, 2, 128], FP8, tag="w2r")
        nc.sync.dma_start(w2r[:], w2r_d[:])
        w2u = cst.tile([128, 2, 128], FP8, tag="w2u")
        nc.sync.dma_start(w2u[:], w2u_d[:])
        wir8 = cst.tile([128, 128], FP8, tag="wir8")
        nc.sync.dma_start(wir8[:], wir_d[:])
        wiu8 = cst.tile([128, 128], FP8, tag="wiu8")
        nc.sync.dma_start(wiu8[:], wiu_d[:])
        w_inT = cst.tile([128, 128], BF16, tag="w_inT")
        nc.sync.dma_start(w_inT[:], win_d[:])
        w_hhnT = cst.tile([128, 128], BF16, tag="w_hhnT")
        nc.scalar.dma_start(w_hhnT[:], whn_d[:])
        fcw2 = cst.tile([128, 2, 128], FP8, tag="fcw2")
        nc.scalar.dma_start(fcw2[:], fcw2_d[:])
        ident = cst.tile([128, 128], BF16, tag="ident")
        nc.scalar.dma_start(ident[:], ident_d[:])
        adj_dr = cst.tile([128, 3, 2, NPAD8], FP8, tag="adj_dr")
        nc.scalar.dma_start(adj_dr[:], adjdr_d[:])
        out_wT6 = cst.tile([128, 6, 128], BF16, tag="out_wT6")
        nc.scalar.dma_start(out_wT6[:], outw_d[:])
        b_r = cst.tile([HID, 1], F32, tag="b_r")
        nc.scalar.dma_start(b_r[:], br_d[:])
        b_u_neg = cst.tile([HID, 1], F32, tag="b_u_neg")
        nc.scalar.dma_start(b_u_neg[:], bun_d[:])
        b_in = cst.tile([HID, 1], F32, tag="b_in")
        nc.scalar.dma_start(b_in[:], bin_d[:])
        b_hn_s = cst.tile([HID, 1], F32, tag="b_hn_s")
        nc.scalar.dma_start(b_hn_s[:], bhn_d[:])

        # warm the ACT function tables early
        dummy = cst.tile([1, 16], F32, tag="dummy")
        nc.scalar.activation(dummy[:], dummy[:], AF.Sigmoid)
        nc.scalar.activation(dummy[:], dummy[:], AF.Tanh)

        # HAM pre-warm: junk matmuls with no DMA dependency at program
        # start (wtile via gpsimd memset finishes earliest)
        wtile = cst.tile([128, 128], BF16, tag="wtile")
        nc.gpsimd.memset(wtile[:], 1.0)
        # dedicated warm bank: no pool rotation, warm matmuls never wait
        ps_warm = ps.tile([128, 512], F32, tag="warm", name="ps_warm", bufs=1)
        for _ in range(96):
            nc.tensor.matmul(ps_warm[:, 0:128], wtile[:], wtile[:],
                             start=True, stop=True)

        # ---- state ----
        h0 = st.tile([HID, TOK], BF16, tag="h0")
        h1 = st.tile([HID, TOK], BF16, tag="h1")
        # S3[par]: fp8 planes (zd8(t), h8(t), x8(t+1)) for t%2 == par
        s3a = st.tile([128, 3, TOK], FP8, tag="s3a")
        s3b = st.tile([128, 3, TOK], FP8, tag="s3b")
        z_fm = st.tile([HID, TOKP], BF16, tag="z_fm")
        zt_all = st.tile([128, BL * NCH, 128], BF16, tag="zt_all")
        zt8 = st.tile([128, BL * NCH, 128], FP8, tag="zt8")
        hbuf = [h0, h1]
        sbuf3 = [s3a, s3b]

        # zero fp8 pad lanes read by matmuls:
        #  - x plane rows 17..127 (fc stationary slot has 0 weights there,
        #    but 0 * NaN would poison the MAC)
        #  - zt_all pad rows of each 6th chunk (74-row tail); zt8 inherits
        #    zeros through the cast
        for s3 in sbuf3:
            nc.gpsimd.memset(s3[:, 2, :], 0.0)
        nc.gpsimd.memset(z_fm[:, TOK:TOKP], 0.0)

        # zd0 (host-exact, fp8 x64) into S3[0] plane 0; x8(1) into S3[1]
        nc.sync.dma_start(s3a[:, 0, :], zd0_d[:])
        nc.sync.dma_start(s3b[0:17, 2, :], x8_d[0])

        def warm(n=1):
            # cheap dependency-free matmuls: keep the PE activity monitor
            # hot across short waits (cold K=4/8 doubles matmul latency)
            for _ in range(n):
                nc.tensor.matmul(ps_warm[:, 0:64], wtile[:], wtile[:, 0:64],
                                 start=True, stop=True)

        def gru_a(t, p, s3c):
            """r and u gates for pair p: DR-fused matmuls + sigmoids."""
            first = t == 0
            s0 = PO[p]
            L = PS[p]
            ps_r = ps.tile([128, 1024], F32, tag="blk", name="ps_r")
            ps_u = ps.tile([128, 1024], F32, tag="blk", name="ps_u")
            warm(2)
            for (o0, o1) in ((0, 512), (512, L)):
                if first:
                    nc.tensor.matmul(ps_r[:, o0:o1], wir8[:],
                                     s3c[:, 0, s0 + o0:s0 + o1],
                                     start=True, stop=True)
                    nc.tensor.matmul(ps_u[:, o0:o1], wiu8[:],
                                     s3c[:, 0, s0 + o0:s0 + o1],
                                     start=True, stop=True)
                else:
                    nc.tensor.matmul(ps_r[:, o0:o1], w2r[:],
                                     s3c[:, 0:2, s0 + o0:s0 + o1],
                                     start=True, stop=True, perf_mode=DR)
                    nc.tensor.matmul(ps_u[:, o0:o1], w2u[:],
                                     s3c[:, 0:2, s0 + o0:s0 + o1],
                                     start=True, stop=True, perf_mode=DR)
            r = gb.tile([128, 960], BF16, tag="r", name="r")
            nc.scalar.activation(r[:, 0:L], ps_r[:, 0:L], AF.Sigmoid,
                                 bias=b_r[:], scale=1.0 / KG)
            v = gb.tile([128, 960], BF16, tag="v", name="v")
            nc.scalar.activation(v[:, 0:L], ps_u[:, 0:L], AF.Sigmoid,
                                 bias=b_u_neg[:], scale=-1.0 / KG)
            return r, v

        def gru_b(t, p, s3c, hc, hn, s3n, rv):
            """n-gate + state update for pair p."""
            first = t == 0
            r, v = rv
            s0 = PO[p]
            L = PS[p]
            t1 = gb.tile([128, 960], BF16, tag="t1", name="t1")
            ps_b = ps.tile([128, 1024], F32, tag="blk", name="ps_b")
            warm(1)
            if first:
                # t1 = r * (1024*b_hn)
                nc.vector.scalar_tensor_tensor(t1[:, 0:L], r[:, 0:L],
                                               b_hn_s[:], r[:, 0:L],
                                               ALU.mult, ALU.bypass)
            else:
                for (o0, o1) in ((0, 512), (512, L)):
                    nc.tensor.matmul(ps_b[:, o0:o1], w_hhnT[:],
                                     hc[:, s0 + o0:s0 + o1],
                                     start=True, stop=True)
                nc.vector.scalar_tensor_tensor(t1[:, 0:L], ps_b[:, 0:L],
                                               b_hn_s[:], r[:, 0:L],
                                               ALU.add, ALU.mult)
            # i_n (mixed fp8 x bf16) + identity-accumulated t1
            for (o0, o1) in ((0, 512), (512, L)):
                nc.tensor.matmul(ps_b[:, o0:o1], w_inT[:],
                                 s3c[:, 0, s0 + o0:s0 + o1],
                                 start=True, stop=False)
                nc.tensor.matmul(ps_b[:, o0:o1], ident[:],
                                 t1[:, o0:o1], start=False, stop=True)
            c = gb.tile([128, 960], BF16, tag="c", name="c")
            nc.scalar.activation(c[:, 0:L], ps_b[:, 0:L], AF.Tanh,
                                 bias=b_in[:], scale=1.0 / KG)
            if first:
                nc.vector.tensor_tensor(hn[:, s0:s0 + L], v[:, 0:L],
                                        c[:, 0:L], ALU.mult)
            else:
                d = gb.tile([128, 960], BF16, tag="d", name="d")
                nc.vector.tensor_tensor(d[:, 0:L], c[:, 0:L],
                                        hc[:, s0:s0 + L], ALU.subtract)
                e = gb.tile([128, 960], BF16, tag="e", name="e")
                nc.vector.tensor_tensor(e[:, 0:L], v[:, 0:L], d[:, 0:L],
                                        ALU.mult)
                nc.vector.tensor_tensor(hn[:, s0:s0 + L], hc[:, s0:s0 + L],
                                        e[:, 0:L], ALU.add)
            # h8 for next step's fc / GRU
            nc.gpsimd.dma_start(s3n[:, 1, s0:s0 + L], hn[:, s0:s0 + L])

        def fc_pair(q, s3n):
            """fc for step t+1, pair q: DR-fused [h8; x8] -> z_fm (x16)."""
            s0 = PO[q]
            L = PS[q]
            ps_z = ps.tile([128, 1024], F32, tag="blk", name="ps_z")
            warm(1)
            for (o0, o1) in ((0, 512), (512, L)):
                nc.tensor.matmul(ps_z[:, o0:o1], fcw2[:],
                                 s3n[:, 1:3, s0 + o0:s0 + o1],
                                 start=True, stop=True, perf_mode=DR)
            nc.scalar.activation(z_fm[:, s0:s0 + L], ps_z[:, 0:L], AF.Relu,
                                 scale=SZ / KG)

        def transpose_b(b):
            """xbar-transpose z window b into zt_all chunks [6b..6b+5]."""
            base = N * b
            nc.sync.dma_start(zt_all[:, NCH * b:NCH * b + 6, :],
                              z_fm[:, base:base + 768], transpose=True)

        def cast_b(b):
            nc.gpsimd.dma_start(zt8[:, NCH * b:NCH * b + 6, :],
                                zt_all[:, NCH * b:NCH * b + 6, :])

        def diffusion_mm(b):
            psd = ps.tile([128, 1024], F32, tag="blk", name="psd")
            warm(1)
            for k2 in range(3):
                lhsT = zt8[:, NCH * b + 2 * k2: NCH * b + 2 * k2 + 2, :]
                stt = k2 == 0
                spp = k2 == 2
                nc.tensor.matmul(psd[:, 0:512], lhsT,
                                 adj_dr[:, k2, :, 0:512],
                                 start=stt, stop=spp, perf_mode=DR)
                nc.tensor.matmul(psd[:, 512:714], lhsT,
                                 adj_dr[:, k2, :, 512:714],
                                 start=stt, stop=spp, perf_mode=DR)
            return psd

        def diffusion_copy(b, psd, s3t):
            base = N * b
            nc.vector.tensor_scalar_mul(s3t[:, 0, base:base + N],
                                        psd[:, 0:714], SZD / (SZ * S_ADJ))

        def diffusion_b(b, s3t):
            """zd8 window b: fp8 DR over 3 chunk-pairs, write fp8 x64."""
            diffusion_copy(b, diffusion_mm(b), s3t)

        def readout_pair(p, hF, qsel):
            """yT[:, pair p] = out_w slices (stationary) x h (moving)."""
            s0 = PO[p]
            L = PS[p]
            for j in range(6):
                rows = 128 if j < 5 else N - 640
                ps_y = ps.tile([128, 1024], F32, tag="blk", name="ps_y")
                for (o0, o1) in ((0, 512), (512, L)):
                    nc.tensor.matmul(ps_y[0:rows, o0:o1],
                                     out_wT6[:, j, 0:rows],
                                     hF[:, s0 + o0:s0 + o1],
                                     start=True, stop=True)
                y_sb = ysb_p.tile([128, 960], BF16, tag="y_sb", name="y_sb")
                if j % 2 == 0:
                    nc.scalar.activation(y_sb[0:rows, 0:L], ps_y[0:rows, 0:L],
                                         AF.Copy)
                else:
                    nc.vector.tensor_copy(y_sb[0:rows, 0:L], ps_y[0:rows, 0:L])
                qeng = (nc.sync, nc.scalar)[qsel % 2]
                qsel += 1
                qeng.dma_start(y_d[128 * j:128 * j + rows, s0:s0 + L],
                               y_sb[0:rows, 0:L])
            return qsel

        # ---- main loop ----
        qsel = 0
        w3_carry = None
        for t in range(T):
            first = t == 0
            last = t == T - 1
            hc, hn = hbuf[t % 2], hbuf[(t + 1) % 2]
            s3c, s3n = sbuf3[t % 2], sbuf3[(t + 1) % 2]
            # x8(t+2) into S3[(t+2)%2] plane 2 (used by fc during step t+1)
            if not last and t + 2 <= T - 1:
                nc.sync.dma_start(sbuf3[(t + 2) % 2][0:17, 2, :],
                                  x8_d[t + 1])
            rv = [None] * NPAIR
            for p in range(NPAIR):
                rv[p] = gru_a(t, p, s3c)
                if p == 0 and w3_carry is not None:
                    # window-3 diffusion copy deferred across the step edge
                    diffusion_copy(3, w3_carry, s3c)
                    w3_carry = None
                if p >= 1:
                    gru_b(t, p - 1, s3c, hc, hn, s3n, rv[p - 1])
                    if last:
                        qsel = readout_pair(p - 1, hn, qsel)
                if not first and p <= 3:
                    diffusion_b(4 + p, s3c)
                if not last:
                    if p >= 2:
                        fc_pair(p - 2, s3n)
                    if p == 3:
                        transpose_b(0)
                        cast_b(0)
                    if p == 4:
                        transpose_b(1)
                        cast_b(1)
                        transpose_b(2)
                        cast_b(2)
                        diffusion_b(0, s3n)
                    if p == 5:
                        transpose_b(3)
                        cast_b(3)
                        diffusion_b(1, s3n)
                        diffusion_b(2, s3n)
            gru_b(t, NPAIR - 1, s3c, hc, hn, s3n, rv[NPAIR - 1])
            if last:
                qsel = readout_pair(NPAIR - 1, hn, qsel)
            else:
                fc_pair(4, s3n)
                transpose_b(4)
                cast_b(4)
                transpose_b(5)
                cast_b(5)
                fc_pair(5, s3n)
                transpose_b(6)
                cast_b(6)
                transpose_b(7)
                cast_b(7)
                w3_carry = diffusion_mm(3)

    nc.compile()
    _NC_CACHE["nc"] = nc
    return nc


def _prep_core_inputs(x_core, shared, fc_w, fc_b, adj):
    m = dict(shared)
    # x8[t-1]: rows 0..15 = x[:, t].T * 16 (feature-major tight tokens),
    # row 16 = 16.0 (bias row)
    x8 = np.zeros((T - 1, 17, TOK), np.float32)
    xt = x_core.transpose(1, 3, 0, 2).reshape(T, D, TOK)  # [t, d, tok]
    x8[:, 0:16, :] = xt[1:T] * SX
    x8[:, 16, :] = SX
    m["x8"] = x8.astype(ml_dtypes.float8_e4m3fn)
    # zd(0) host-exact: adj-diffusion of relu(fc_x(x_0)), fp8 x64
    z0 = np.maximum(x_core[:, 0] @ fc_w[:, :D].T + fc_b, 0.0)  # [BL, N, HID]
    zd0 = np.matmul(adj, z0) * SZD                             # [BL, N, HID]
    m["zd0"] = np.ascontiguousarray(
        zd0.reshape(TOK, HID).T).astype(ml_dtypes.float8_e4m3fn)
    return m


def run(inputs, trace=False):
    x = np.asarray(inputs["x"], np.float32)
    adj = np.asarray(inputs["adj"], np.float32)
    fc_w = np.asarray(inputs["fc_w"], np.float32)
    fc_b = np.asarray(inputs["fc_b"], np.float32)
    w_ih = np.asarray(inputs["w_ih"], np.float32)
    w_hh = np.asarray(inputs["w_hh"], np.float32)
    b_ih = np.asarray(inputs["b_ih"], np.float32)
    b_hh = np.asarray(inputs["b_hh"], np.float32)
    out_w = np.asarray(inputs["out_w"], np.float32)
    out_b = np.asarray(inputs["out_b"], np.float32)
    f8 = ml_dtypes.float8_e4m3fn

    # diffusion operand: adj_dr[p, k2, ko, n] = adj[n, 256*k2+128*ko+p]*S_ADJ
    adjT = np.zeros((768, NPAD8), np.float32)
    adjT[:N, :N] = adj.T * S_ADJ
    adj_dr = np.ascontiguousarray(
        adjT.reshape(3, 2, 128, NPAD8).transpose(2, 0, 1, 3)).astype(f8)

    wihT = w_ih.T.copy()   # [HID, 3H]
    whhT = w_hh.T.copy()

    def dr_pack(wi_slice, wh_slice):
        w2 = np.empty((128, 2, 128), np.float32)
        w2[:, 0, :] = wi_slice * SWI
        w2[:, 1, :] = wh_slice * SWH
        return w2.astype(f8)

    fcw2 = np.zeros((128, 2, 128), np.float32)
    fcw2[:, 0, :] = fc_w[:, D:].T * SWH          # h slot
    fcw2[0:16, 1, :] = fc_w[:, :D].T * SWX       # x slot
    fcw2[16, 1, :] = fc_b * SWX                  # bias row (x8 row16 = 16)

    outw_p = np.zeros((128, 6, 128), np.float32)
    outw_p.reshape(128, 768)[:, :N] = out_w.T

    shared = {
        "adj_dr": adj_dr,
        "w2r": dr_pack(wihT[:, 0:128], whhT[:, 0:128]),
        "w2u": dr_pack(wihT[:, 128:256], whhT[:, 128:256]),
        "wir8": (wihT[:, 0:128] * SWI).astype(f8),
        "wiu8": (wihT[:, 128:256] * SWI).astype(f8),
        "w_inT": (wihT[:, 256:384] * SWI).astype(ml_dtypes.bfloat16),
        "w_hhnT": (whhT[:, 256:384] * SWH).astype(ml_dtypes.bfloat16),
        "fcw2": fcw2.astype(f8),
        "ident": np.eye(128, dtype=np.float32).astype(ml_dtypes.bfloat16),
        "out_wT6": outw_p.astype(ml_dtypes.bfloat16),
        "b_r": (b_ih[0:128] + b_hh[0:128]).reshape(HID, 1).astype(np.float32),
        "b_u_neg": (-(b_ih[128:256] + b_hh[128:256])).reshape(HID, 1).astype(np.float32),
        "b_in": b_ih[256:384].reshape(HID, 1).astype(np.float32),
        "b_hn_s": (b_hh[256:384] * KG).reshape(HID, 1).astype(np.float32),
    }

    nc = _build_program()
    in_maps = [_prep_core_inputs(x[BL * i: BL * (i + 1)], shared, fc_w, fc_b, adj)
               for i in range(CORES)]
    res = run_bass_kernel_spmd(nc, in_maps, list(range(CORES)), trace=trace)
    # yT[j, 714b+n] -> y[b, n, j], bias added here
    ys = []
    for i in range(CORES):
        yT = np.asarray(res.results[i]["yT"]).astype(np.float32)
        ys.append(yT.reshape(N, BL, N).transpose(1, 2, 0) + out_b[None, None, :])
    return np.concatenate(ys, axis=0), res


def kernel(**inputs) -> np.ndarray:
    y, _ = run(inputs, trace=False)
    return y


# revision 11
# speedup vs baseline: 1.1702x; 1.1702x over previous
"""DCRNN kernel for Trainium2 (8 NeuronCores, data-parallel over batch).

Model (per time step t, 6 steps):
    z  = relu([x_t, h] @ fc_w.T + fc_b)          # [b, n, 128]
    zd = einsum('nm,bmh->bnh', adj, z)           # graph diffusion
    GRU(zd, h) -> h                              # gated update
Readout: y = h @ out_w.T + out_b                 # [b, n, 714]

v3 design (per core, batch shard of 8, tokens packed TIGHT = 8*714 = 5712):
  - fp8 DoubleRow everywhere the contraction allows C=256 fusion:
      fc:   [h8; x8] planes vs [fc_wh x1024; fc_wx x64 + bias row] slots
      GRU:  r/u gates fused (ih+hh) via [zd8; h8] planes
      diffusion: z.T fp8 chunks vs adj fp8 (as v2)
  - scales: zd8 = zd*64, h8 = h*1, x8 = x*16, all gate PSUMs = 1024x true
  - i_n: mixed-dtype matmul (fp8 zd8 moving x bf16 w_in*16 stationary)
  - t1 = (h_n + b_hn)*r accumulated into i_n's PSUM via identity matmul;
    tanh reads PSUM directly (no sg2 DVE op)
  - h' = h + v*(c-h): d/e on DVE, h-copy + accumulate + h8-cast on the
    gpsimd DMA queue (0 DVE cost)
  - token pairs of 960/912 (16B-aligned for fp8 DR), matmuls split 512+rest
    at PSUM bank boundaries, sigma/tanh/relu ACTs span 2 banks (one op/pair)
  - readout flipped: yT[j, tok] with out_w slices stationary (LDW amortized),
    plain copies (bias added on host), DMAs spread across step-5 pairs
"""
import sys
import types

sys.path.insert(0, "/opt/trn_rl_repo")

import numpy as np
import ml_dtypes
from contextlib import ExitStack

# NTFF profile hook shim: the agent image lacks antenv.axon_hooks; provide it
# so run_bass_kernel_spmd(trace=True) can profile. Harmless when unused.
try:
    import antenv.axon_hooks  # noqa: F401
except ImportError:
    try:
        import trn_agent_boot.trn_boot as _tb

        _m = types.ModuleType("antenv.axon_hooks")
        _hook = _tb._ntff_profile_via_ctypes("/opt/axon/libaxon_pjrt.so")
        _m.get_axon_ntff_profile_hook = lambda: _hook
        _m.set_axon_ntff_profile_hook = lambda h: None
        sys.modules["antenv.axon_hooks"] = _m
    except Exception:
        pass

from concourse import bacc, tile, mybir
from concourse.bass_utils import run_bass_kernel_spmd

F32 = mybir.dt.float32
BF16 = mybir.dt.bfloat16
FP8 = mybir.dt.float8e4
AF = mybir.ActivationFunctionType
ALU = mybir.AluOpType
DR = mybir.MatmulPerfMode.DoubleRow

B, T, N, D, HID = 64, 6, 714, 16, 128
CORES = 8
BL = B // CORES            # batch per core
TOK = BL * N               # 5712 tight tokens
TOKP = TOK + 64            # z_fm pad tail so window-7 transpose stays in-bounds
NPAD8 = 720                # adj_dr inner pad (16-aligned)
NCH = 6                    # m-chunks per batch item (5x128 + 74)

# token pairs: 16B-aligned starts, PSUM 2-bank (1024 f32) tiles
PO = [0, 960, 1920, 2880, 3840, 4800]
PS = [960, 960, 960, 960, 960, 912]
NPAIR = 6

S_ADJ = 1024.0             # adj stored x1024 in fp8
SZ = 16.0                  # z stored x16 (z_fm bf16 and zt8 fp8)
SZD = 64.0                 # zd stored x64 in fp8
SWI = 16.0                 # w_ih (r/u/n slots vs zd8) x16
SWH = 1024.0               # w_hh (vs h8 x1) x1024
SX = 16.0                  # x plane x16
SWX = 64.0                 # fc x-weights x64
KG = 1024.0                # every gate PSUM = KG * true

_NC_CACHE = {}


def _build_program():
    if "nc" in _NC_CACHE:
        return _NC_CACHE["nc"]

    nc = bacc.Bacc(
        "TRN2",
        target_bir_lowering=False,
        debug=False,
        enable_asserts=True,
        num_devices=CORES,
    )

    x8_d = nc.declare_dram_parameter("x8", [T - 1, 17, TOK], FP8, isOutput=False)
    zd0_d = nc.declare_dram_parameter("zd0", [HID, TOK], FP8, isOutput=False)
    adjdr_d = nc.declare_dram_parameter("adj_dr", [128, 3, 2, NPAD8], FP8, isOutput=False)
    w2r_d = nc.declare_dram_parameter("w2r", [128, 2, 128], FP8, isOutput=False)
    w2u_d = nc.declare_dram_parameter("w2u", [128, 2, 128], FP8, isOutput=False)
    wir_d = nc.declare_dram_parameter("wir8", [128, 128], FP8, isOutput=False)
    wiu_d = nc.declare_dram_parameter("wiu8", [128, 128], FP8, isOutput=False)
    win_d = nc.declare_dram_parameter("w_inT", [128, 128], BF16, isOutput=False)
    whn_d = nc.declare_dram_parameter("w_hhnT", [128, 128], BF16, isOutput=False)
    fcw2_d = nc.declare_dram_parameter("fcw2", [128, 2, 128], FP8, isOutput=False)
    ident_d = nc.declare_dram_parameter("ident", [128, 128], BF16, isOutput=False)
    outw_d = nc.declare_dram_parameter("out_wT6", [128, 6, 128], BF16, isOutput=False)
    br_d = nc.declare_dram_parameter("b_r", [HID, 1], F32, isOutput=False)
    bun_d = nc.declare_dram_parameter("b_u_neg", [HID, 1], F32, isOutput=False)
    bin_d = nc.declare_dram_parameter("b_in", [HID, 1], F32, isOutput=False)
    bhn_d = nc.declare_dram_parameter("b_hn_s", [HID, 1], F32, isOutput=False)
    y_d = nc.declare_dram_parameter("yT", [N, TOK], BF16, isOutput=True)

    with tile.TileContext(nc) as tc, ExitStack() as ctx:
        cst = ctx.enter_context(tc.tile_pool(name="cst", bufs=1))
        st = ctx.enter_context(tc.tile_pool(name="st", bufs=1))
        gb = ctx.enter_context(tc.tile_pool(name="gb", bufs=4))
        ysb_p = ctx.enter_context(tc.tile_pool(name="ysb", bufs=8))
        ps = ctx.enter_context(tc.tile_pool(name="ps", bufs=2, space="PSUM"))

        # ---- constants in (t=0-critical first) ----
        zd0 = None  # DMA'd into S3 plane later (need tiles declared first)
        w2r = cst.tile([128, 2, 128], FP8, tag="w2r")
        nc.sync.dma_start(w2r[:], w2r_d[:])
        w2u = cst.tile([128, 2, 128], FP8, tag="w2u")
        nc.sync.dma_start(w2u[:], w2u_d[:])
        wir8 = cst.tile([128, 128], FP8, tag="wir8")
        nc.sync.dma_start(wir8[:], wir_d[:])
        wiu8 = cst.tile([128, 128], FP8, tag="wiu8")
        nc.sync.dma_start(wiu8[:], wiu_d[:])
        w_inT = cst.tile([128, 128], BF16, tag="w_inT")
        nc.sync.dma_start(w_inT[:], win_d[:])
        w_hhnT = cst.tile([128, 128], BF16, tag="w_hhnT")
        nc.scalar.dma_start(w_hhnT[:], whn_d[:])
        fcw2 = cst.tile([128, 2, 128], FP8, tag="fcw2")
        nc.scalar.dma_start(fcw2[:], fcw2_d[:])
        ident = cst.tile([128, 128], BF16, tag="ident")
        nc.scalar.dma_start(ident[:], ident_d[:])
        adj_dr = cst.tile([128, 3, 2, NPAD8], FP8, tag="adj_dr")
        nc.scalar.dma_start(adj_dr[:], adjdr_d[:])
        out_wT6 = cst.tile([128, 6, 128], BF16, tag="out_wT6")
        nc.scalar.dma_start(out_wT6[:], outw_d[:])
        b_r = cst.tile([HID, 1], F32, tag="b_r")
        nc.scalar.dma_start(b_r[:], br_d[:])
        b_u_neg = cst.tile([HID, 1], F32, tag="b_u_neg")
        nc.scalar.dma_start(b_u_neg[:], bun_d[:])
        b_in = cst.tile([HID, 1], F32, tag="b_in")
        nc.scalar.dma_start(b_in[:], bin_d[:])
        b_hn_s = cst.tile([HID, 1], F32, tag="b_hn_s")
        nc.scalar.dma_start(b_hn_s[:], bhn_d[:])

        # warm the ACT function tables early
        dummy = cst.tile([1, 16], F32, tag="dummy")
        nc.scalar.activation(dummy[:], dummy[:], AF.Sigmoid)
        nc.scalar.activation(dummy[:], dummy[:], AF.Tanh)

        # HAM pre-warm: junk matmuls with no DMA dependency at program
        # start (wtile via gpsimd memset finishes earliest)
        wtile = cst.tile([128, 128], BF16, tag="wtile")
        nc.gpsimd.memset(wtile[:], 1.0)
        ps_j = ps.tile([128, 1024], F32, tag="blk", name="ps_j")
        for _ in range(96):
            nc.tensor.matmul(ps_j[:, 0:128], wtile[:], wtile[:],
                             start=True, stop=True)

        # ---- state ----
        h0 = st.tile([HID, TOK], BF16, tag="h0")
        h1 = st.tile([HID, TOK], BF16, tag="h1")
        # S3[par]: fp8 planes (zd8(t), h8(t), x8(t+1)) for t%2 == par
        s3a = st.tile([128, 3, TOK], FP8, tag="s3a")
        s3b = st.tile([128, 3, TOK], FP8, tag="s3b")
        z_fm = st.tile([HID, TOKP], BF16, tag="z_fm")
        zt_all = st.tile([128, BL * NCH, 128], BF16, tag="zt_all")
        zt8 = st.tile([128, BL * NCH, 128], FP8, tag="zt8")
        hbuf = [h0, h1]
        sbuf3 = [s3a, s3b]

        # zero fp8 pad lanes read by matmuls:
        #  - x plane rows 17..127 (fc stationary slot has 0 weights there,
        #    but 0 * NaN would poison the MAC)
        #  - zt_all pad rows of each 6th chunk (74-row tail); zt8 inherits
        #    zeros through the cast
        for s3 in sbuf3:
            nc.gpsimd.memset(s3[:, 2, :], 0.0)
        nc.gpsimd.memset(z_fm[:, TOK:TOKP], 0.0)

        # zd0 (host-exact, fp8 x64) into S3[0] plane 0; x8(1) into S3[1]
        nc.sync.dma_start(s3a[:, 0, :], zd0_d[:])
        nc.sync.dma_start(s3b[0:17, 2, :], x8_d[0])

        def gru_a(t, p, s3c, hc):
            """pair p stage A: r/u (+h_n) matmuls, sigmoids, t1."""
            first = t == 0
            s0 = PO[p]
            L = PS[p]
            ps_r = ps.tile([128, 1024], F32, tag="blk", name="ps_r")
            ps_u = ps.tile([128, 1024], F32, tag="blk", name="ps_u")
            for (o0, o1) in ((0, 512), (512, L)):
                if first:
                    nc.tensor.matmul(ps_r[:, o0:o1], wir8[:],
                                     s3c[:, 0, s0 + o0:s0 + o1],
                                     start=True, stop=True)
                    nc.tensor.matmul(ps_u[:, o0:o1], wiu8[:],
                                     s3c[:, 0, s0 + o0:s0 + o1],
                                     start=True, stop=True)
                else:
                    nc.tensor.matmul(ps_r[:, o0:o1], w2r[:],
                                     s3c[:, 0:2, s0 + o0:s0 + o1],
                                     start=True, stop=True, perf_mode=DR)
                    nc.tensor.matmul(ps_u[:, o0:o1], w2u[:],
                                     s3c[:, 0:2, s0 + o0:s0 + o1],
                                     start=True, stop=True, perf_mode=DR)
            ps_b = ps.tile([128, 1024], F32, tag="bb", name="ps_b")
            if not first:
                for (o0, o1) in ((0, 512), (512, L)):
                    nc.tensor.matmul(ps_b[:, o0:o1], w_hhnT[:],
                                     hc[:, s0 + o0:s0 + o1],
                                     start=True, stop=True)
            r = gb.tile([128, 960], BF16, tag="r", name="r")
            nc.scalar.activation(r[:, 0:L], ps_r[:, 0:L], AF.Sigmoid,
                                 bias=b_r[:], scale=1.0 / KG)
            v = gb.tile([128, 960], BF16, tag="v", name="v")
            nc.scalar.activation(v[:, 0:L], ps_u[:, 0:L], AF.Sigmoid,
                                 bias=b_u_neg[:], scale=-1.0 / KG)
            t1 = gb.tile([128, 960], BF16, tag="t1", name="t1")
            if first:
                nc.vector.scalar_tensor_tensor(t1[:, 0:L], r[:, 0:L],
                                               b_hn_s[:], r[:, 0:L],
                                               ALU.mult, ALU.bypass)
            else:
                nc.vector.scalar_tensor_tensor(t1[:, 0:L], ps_b[:, 0:L],
                                               b_hn_s[:], r[:, 0:L],
                                               ALU.add, ALU.mult)
            return ps_b, v, t1

        def gru_b(t, p, s3c, hc, hn, s3n, avt):
            """pair p stage B: i_n + t1 accumulate, tanh, state update."""
            first = t == 0
            ps_b, v, t1 = avt
            s0 = PO[p]
            L = PS[p]
            for (o0, o1) in ((0, 512), (512, L)):
                nc.tensor.matmul(ps_b[:, o0:o1], w_inT[:],
                                 s3c[:, 0, s0 + o0:s0 + o1],
                                 start=True, stop=False)
                nc.tensor.matmul(ps_b[:, o0:o1], ident[:],
                                 t1[:, o0:o1], start=False, stop=True)
            c = gb.tile([128, 960], BF16, tag="c", name="c")
            nc.scalar.activation(c[:, 0:L], ps_b[:, 0:L], AF.Tanh,
                                 bias=b_in[:], scale=1.0 / KG)
            if first:
                nc.vector.tensor_tensor(hn[:, s0:s0 + L], v[:, 0:L],
                                        c[:, 0:L], ALU.mult)
            else:
                d = gb.tile([128, 960], BF16, tag="d", name="d")
                nc.vector.tensor_tensor(d[:, 0:L], c[:, 0:L],
                                        hc[:, s0:s0 + L], ALU.subtract)
                e = gb.tile([128, 960], BF16, tag="e", name="e")
                nc.vector.tensor_tensor(e[:, 0:L], v[:, 0:L], d[:, 0:L],
                                        ALU.mult)
                nc.vector.tensor_tensor(hn[:, s0:s0 + L], hc[:, s0:s0 + L],
                                        e[:, 0:L], ALU.add)
            # h8 for next step's fc / GRU
            nc.gpsimd.dma_start(s3n[:, 1, s0:s0 + L], hn[:, s0:s0 + L])

        def fc_pair(q, s3n):
            """fc for step t+1, pair q: DR-fused [h8; x8] -> z_fm (x16)."""
            s0 = PO[q]
            L = PS[q]
            ps_z = ps.tile([128, 1024], F32, tag="blk", name="ps_z")
            for (o0, o1) in ((0, 512), (512, L)):
                nc.tensor.matmul(ps_z[:, o0:o1], fcw2[:],
                                 s3n[:, 1:3, s0 + o0:s0 + o1],
                                 start=True, stop=True, perf_mode=DR)
            nc.scalar.activation(z_fm[:, s0:s0 + L], ps_z[:, 0:L], AF.Relu,
                                 scale=SZ / KG)

        def transpose_b(b):
            """xbar-transpose z window b into zt_all chunks [6b..6b+5]."""
            base = N * b
            nc.sync.dma_start(zt_all[:, NCH * b:NCH * b + 6, :],
                              z_fm[:, base:base + 768], transpose=True)

        def cast_b(b):
            nc.gpsimd.dma_start(zt8[:, NCH * b:NCH * b + 6, :],
                                zt_all[:, NCH * b:NCH * b + 6, :])

        def diffusion_mm(b):
            psd = ps.tile([128, 1024], F32, tag="blk", name="psd")
            for k2 in range(3):
                lhsT = zt8[:, NCH * b + 2 * k2: NCH * b + 2 * k2 + 2, :]
                stt = k2 == 0
                spp = k2 == 2
                nc.tensor.matmul(psd[:, 0:512], lhsT,
                                 adj_dr[:, k2, :, 0:512],
                                 start=stt, stop=spp, perf_mode=DR)
                nc.tensor.matmul(psd[:, 512:714], lhsT,
                                 adj_dr[:, k2, :, 512:714],
                                 start=stt, stop=spp, perf_mode=DR)
            return psd

        def diffusion_copy(b, psd, s3t):
            base = N * b
            nc.vector.tensor_scalar_mul(s3t[:, 0, base:base + N],
                                        psd[:, 0:714], SZD / (SZ * S_ADJ))

        def diffusion_b(b, s3t):
            """zd8 window b: fp8 DR over 3 chunk-pairs, write fp8 x64."""
            diffusion_copy(b, diffusion_mm(b), s3t)

        def readout_pair(p, hF, qsel):
            """yT[:, pair p] = out_w slices (stationary) x h (moving)."""
            s0 = PO[p]
            L = PS[p]
            for j in range(6):
                rows = 128 if j < 5 else N - 640
                ps_y = ps.tile([128, 1024], F32, tag="blk", name="ps_y")
                for (o0, o1) in ((0, 512), (512, L)):
                    nc.tensor.matmul(ps_y[0:rows, o0:o1],
                                     out_wT6[:, j, 0:rows],
                                     hF[:, s0 + o0:s0 + o1],
                                     start=True, stop=True)
                y_sb = ysb_p.tile([128, 960], BF16, tag="y_sb", name="y_sb")
                if j % 2 == 0:
                    nc.scalar.activation(y_sb[0:rows, 0:L], ps_y[0:rows, 0:L],
                                         AF.Copy)
                else:
                    nc.vector.tensor_copy(y_sb[0:rows, 0:L], ps_y[0:rows, 0:L])
                qeng = (nc.sync, nc.scalar)[qsel % 2]
                qsel += 1
                qeng.dma_start(y_d[128 * j:128 * j + rows, s0:s0 + L],
                               y_sb[0:rows, 0:L])
            return qsel

        # ---- main loop ----
        qsel = 0
        w3_carry = None
        for t in range(T):
            first = t == 0
            last = t == T - 1
            hc, hn = hbuf[t % 2], hbuf[(t + 1) % 2]
            s3c, s3n = sbuf3[t % 2], sbuf3[(t + 1) % 2]
            # x8(t+2) into S3[(t+2)%2] plane 2 (used by fc during step t+1)
            if not last and t + 2 <= T - 1:
                nc.sync.dma_start(sbuf3[(t + 2) % 2][0:17, 2, :],
                                  x8_d[t + 1])
            rv = [None] * NPAIR
            for p in range(NPAIR):
                if p == 0 and w3_carry is not None:
                    # window-3 diffusion copy deferred across the step edge
                    diffusion_copy(3, w3_carry, s3c)
                    w3_carry = None
                if p >= 1:
                    gru_b(t, p - 1, s3c, hc, hn, s3n, rv[p - 1])
                    if last:
                        qsel = readout_pair(p - 1, hn, qsel)
                rv[p] = gru_a(t, p, s3c, hc)
                if not first and p <= 3:
                    diffusion_b(4 + p, s3c)
                if not last:
                    if p >= 2:
                        fc_pair(p - 2, s3n)
                    if p == 3:
                        transpose_b(0)
                        cast_b(0)
                    if p == 4:
                        transpose_b(1)
                        cast_b(1)
                        transpose_b(2)
                        cast_b(2)
                        diffusion_b(0, s3n)
                    if p == 5:
                        transpose_b(3)
                        cast_b(3)
                        diffusion_b(1, s3n)
                        diffusion_b(2, s3n)
            gru_b(t, NPAIR - 1, s3c, hc, hn, s3n, rv[NPAIR - 1])
            if last:
                qsel = readout_pair(NPAIR - 1, hn, qsel)
            else:
                fc_pair(4, s3n)
                transpose_b(4)
                cast_b(4)
                transpose_b(5)
                cast_b(5)
                fc_pair(5, s3n)
                transpose_b(6)
                cast_b(6)
                transpose_b(7)
                cast_b(7)
                w3_carry = diffusion_mm(3)

    nc.compile()
    _NC_CACHE["nc"] = nc
    return nc


def _prep_core_inputs(x_core, shared, fc_w, fc_b, adj):
    m = dict(shared)
    # x8[t-1]: rows 0..15 = x[:, t].T * 16 (feature-major tight tokens),
    # row 16 = 16.0 (bias row)
    x8 = np.zeros((T - 1, 17, TOK), np.float32)
    xt = x_core.transpose(1, 3, 0, 2).reshape(T, D, TOK)  # [t, d, tok]
    x8[:, 0:16, :] = xt[1:T] * SX
    x8[:, 16, :] = SX
    m["x8"] = x8.astype(ml_dtypes.float8_e4m3fn)
    # zd(0) host-exact: adj-diffusion of relu(fc_x(x_0)), fp8 x64
    z0 = np.maximum(x_core[:, 0] @ fc_w[:, :D].T + fc_b, 0.0)  # [BL, N, HID]
    zd0 = np.matmul(adj, z0) * SZD                             # [BL, N, HID]
    m["zd0"] = np.ascontiguousarray(
        zd0.reshape(TOK, HID).T).astype(ml_dtypes.float8_e4m3fn)
    return m


def run(inputs, trace=False):
    x = np.asarray(inputs["x"], np.float32)
    adj = np.asarray(inputs["adj"], np.float32)
    fc_w = np.asarray(inputs["fc_w"], np.float32)
    fc_b = np.asarray(inputs["fc_b"], np.float32)
    w_ih = np.asarray(inputs["w_ih"], np.float32)
    w_hh = np.asarray(inputs["w_hh"], np.float32)
    b_ih = np.asarray(inputs["b_ih"], np.float32)
    b_hh = np.asarray(inputs["b_hh"], np.float32)
    out_w = np.asarray(inputs["out_w"], np.float32)
    out_b = np.asarray(inputs["out_b"], np.float32)
    f8 = ml_dtypes.float8_e4m3fn

    # diffusion operand: adj_dr[p, k2, ko, n] = adj[n, 256*k2+128*ko+p]*S_ADJ
    adjT = np.zeros((768, NPAD8), np.float32)
    adjT[:N, :N] = adj.T * S_ADJ
    adj_dr = np.ascontiguousarray(
        adjT.reshape(3, 2, 128, NPAD8).transpose(2, 0, 1, 3)).astype(f8)

    wihT = w_ih.T.copy()   # [HID, 3H]
    whhT = w_hh.T.copy()

    def dr_pack(wi_slice, wh_slice):
        w2 = np.empty((128, 2, 128), np.float32)
        w2[:, 0, :] = wi_slice * SWI
        w2[:, 1, :] = wh_slice * SWH
        return w2.astype(f8)

    fcw2 = np.zeros((128, 2, 128), np.float32)
    fcw2[:, 0, :] = fc_w[:, D:].T * SWH          # h slot
    fcw2[0:16, 1, :] = fc_w[:, :D].T * SWX       # x slot
    fcw2[16, 1, :] = fc_b * SWX                  # bias row (x8 row16 = 16)

    outw_p = np.zeros((128, 6, 128), np.float32)
    outw_p.reshape(128, 768)[:, :N] = out_w.T

    shared = {
        "adj_dr": adj_dr,
        "w2r": dr_pack(wihT[:, 0:128], whhT[:, 0:128]),
        "w2u": dr_pack(wihT[:, 128:256], whhT[:, 128:256]),
        "wir8": (wihT[:, 0:128] * SWI).astype(f8),
        "wiu8": (wihT[:, 128:256] * SWI).astype(f8),
        "w_inT": (wihT[:, 256:384] * SWI).astype(ml_dtypes.bfloat16),
        "w_hhnT": (whhT[:, 256:384] * SWH).astype(ml_dtypes.bfloat16),
        "fcw2": fcw2.astype(f8),
        "ident": np.eye(128, dtype=np.float32).astype(ml_dtypes.bfloat16),
        "out_wT6": outw_p.astype(ml_dtypes.bfloat16),
        "b_r": (b_ih[0:128] + b_hh[0:128]).reshape(HID, 1).astype(np.float32),
        "b_u_neg": (-(b_ih[128:256] + b_hh[128:256])).reshape(HID, 1).astype(np.float32),
        "b_in": b_ih[256:384].reshape(HID, 1).astype(np.float32),
        "b_hn_s": (b_hh[256:384] * KG).reshape(HID, 1).astype(np.float32),
    }

    nc = _build_program()
    in_maps = [_prep_core_inputs(x[BL * i: BL * (i + 1)], shared, fc_w, fc_b, adj)
               for i in range(CORES)]
    res = run_bass_kernel_spmd(nc, in_maps, list(range(CORES)), trace=trace)
    # yT[j, 714b+n] -> y[b, n, j], bias added here
    ys = []
    for i in range(CORES):
        yT = np.asarray(res.results[i]["yT"]).astype(np.float32)
        ys.append(yT.reshape(N, BL, N).transpose(1, 2, 0) + out_b[None, None, :])
    return np.concatenate(ys, axis=0), res


def kernel(**inputs) -> np.ndarray:
    y, _ = run(inputs, trace=False)
    return y
